# revision 10
# baseline (speedup 1.0000x reference)
"""Causal self-attention (B=2, S=2048, D=1024, H=16) on 8 TRN2 NeuronCores.

Sharding (Megatron-style): 2 batches x 4 head-groups -> 8 cores.
Core c handles batch b = c // 4 and local heads [4*(c%4), 4*(c%4)+4).

Per-core device program (single NEFF, SPMD with per-core input shards):
  QT = Wq_g.T @ x_b.T          [256, 2048]  (head-dim on partitions)
  KT = Wk_g.T @ x_b.T          [256, 2048]
  V  = x_b @ Wv_g              [2048, 256]  (seq on partitions), augmented
                               with a ones column per head for the softmax
                               denominator.
  per head h, per q-block of 512:
    ST[k,q] = K_h Q_h^T        (transposed scores, k on partitions; two heads
                               run concurrently as 64-row PE tiles)
    E       = exp(ST/8) * causal_mask      (ACT + DVE)
    ctxT[hd+1, q] += V_aug_h[kblock].T @ E (PSUM accumulate over k blocks;
                                            row hd holds the denominator)
    rc   = 1/den               (DVE, PSUM row 64 -> SBUF partition 0)
    bc   = bcast(rc, 64 parts) (GPSIMD partition_broadcast)
    cn2[pair] packs both heads' normalized ctx on 128 partitions (DVE muls
    with cross-partition-base writes)
  y_partial[q, :] = sum_pair cn2_pair.T @ Wo2_pair   (contraction 128,
    software-pipelined one q-block behind attention)
Host sums the 4 partial y's per batch (tensor-parallel reduction on host).
"""

import os
import sys

if "/opt/trn_rl_repo" not in sys.path:
    sys.path.insert(0, "/opt/trn_rl_repo")

from contextlib import ExitStack

import numpy as np

import concourse.bass as bass
import concourse.mybir as mybir
import concourse.tile as tile
from concourse import bacc
from concourse.bass_utils import run_bass_kernel_spmd

B, S, D, H, HD = 2, 2048, 1024, 16, 64
HPC = 4            # heads per core
CD = HPC * HD      # 256: per-core projection width
NCORES = 8
QB = 512           # q block size (one PSUM bank of fp32)
NDT = D // 128     # 8 contraction tiles for projections
NKT = S // 128     # 16 seq tiles
f32 = mybir.dt.float32
bf16 = mybir.dt.bfloat16
EXP = mybir.ActivationFunctionType.Exp


def _attention_qb(nc, qb, QT_sb, KT_sb, V4, masks, pools):
    """Emit scores/exp/mask/PV/norm for one q block; return packed cn2 tiles."""
    stp, accp, ep, rcp, bcp, ctxnp = pools
    cn2s = []
    for pair in range(2):
        QTp, KTp = QT_sb[pair], KT_sb[pair]
        nkt = 4 * (qb + 1)
        ctxA = accp.tile([HD + 1, QB], f32, name="ctxA", tag="acc")
        ctxB = accp.tile([HD + 1, QB], f32, name="ctxB", tag="acc")

        def emit_pv(kt, lo, eT):
            nc.tensor.matmul(
                ctxA[:, lo:QB], V4[:, kt, 2 * pair, :], eT[:, lo:QB],
                start=(kt == 0), stop=(kt == nkt - 1), skip_group_check=True,
            )
            nc.tensor.matmul(
                ctxB[:, lo:QB], V4[:, kt, 2 * pair + 1, :],
                eT[:, QB + lo:2 * QB],
                start=(kt == 0), stop=(kt == nkt - 1), skip_group_check=True,
            )

        pv_pipe = os.environ.get("KERNEL_PV_PIPE") == "1"
        pv_pending = None
        for kt in range(nkt):
            rel = kt - 4 * qb
            # causal band: columns q < rel*128 of this k-tile are fully
            # masked; skip them in ST/mask/PV
            lo = rel * 128 if rel > 0 else 0
            stT = stp.tile([128, 2 * QB], f32, name="stT", tag="st")
            # head A on PE rows 0-63, head B on rows 64-127 (concurrent)
            nc.tensor.matmul(
                stT[:, lo:QB],
                KTp[0:HD, kt * 128:(kt + 1) * 128],
                QTp[0:HD, qb * QB + lo:(qb + 1) * QB],
                start=True, stop=True,
            )
            nc.tensor.matmul(
                stT[:, QB + lo:2 * QB],
                KTp[HD:128, kt * 128:(kt + 1) * 128],
                QTp[HD:128, qb * QB + lo:(qb + 1) * QB],
                start=True, stop=True,
            )
            eT = ep.tile([128, 2 * QB], bf16, name="eT", tag="e")
            if lo == 0:
                nc.scalar.activation(eT, stT, EXP, scale=0.125)
            else:
                nc.scalar.activation(eT[:, lo:QB], stT[:, lo:QB],
                                     EXP, scale=0.125)
                nc.scalar.activation(eT[:, QB + lo:2 * QB],
                                     stT[:, QB + lo:2 * QB],
                                     EXP, scale=0.125)
            if rel >= 0:
                # only the first 128 band columns are partially masked;
                # beyond them every k-row is causal-valid
                msl = masks[:, rel * QB + lo:rel * QB + lo + 128]
                nc.vector.tensor_mul(eT[:, lo:lo + 128],
                                     eT[:, lo:lo + 128], msl)
                nc.vector.tensor_mul(eT[:, QB + lo:QB + lo + 128],
                                     eT[:, QB + lo:QB + lo + 128], msl)
            if pv_pipe:
                # software-pipeline: PV for kt-1 is emitted after scores of
                # kt so the PE runs ahead while ACT computes exp(kt-1)
                if pv_pending is not None:
                    emit_pv(*pv_pending)
                pv_pending = (kt, lo, eT)
            else:
                emit_pv(kt, lo, eT)
        if pv_pending is not None:
            emit_pv(*pv_pending)
        # normalization: pack both heads into one 128-partition tile so the
        # output projection contracts over 128
        cn2 = ctxnp.tile([128, QB], bf16, name=f"cn2_{pair}", tag=f"cn2{pair}")
        for (ctx, half) in ((ctxA, 0), (ctxB, 1)):
            rc = rcp.tile([1, QB], f32, name="rc", tag="rc")
            nc.vector.reciprocal(rc, ctx[HD:HD + 1, :])
            bc = bcp.tile([64, QB], f32, name="bc", tag="bc")
            nc.gpsimd.partition_broadcast(bc, rc)
            nc.vector.tensor_mul(cn2[64 * half:64 * half + 64, :],
                                 ctx[0:HD, :], bc)
        cn2s.append(cn2)
    return cn2s


def _outproj_qb(nc, qb, cn2s, wo_sb, y, ypsp, ysbp):
    for qt in range(QB // 128):
        for nh in range(2):
            yps = ypsp.tile([128, 512], f32, name="yps", tag="yps")
            for pair in range(2):
                nc.tensor.matmul(
                    yps,
                    cn2s[pair][:, qt * 128:(qt + 1) * 128],
                    wo_sb[:, pair, nh * 512:(nh + 1) * 512],
                    start=(pair == 0), stop=(pair == 1),
                    skip_group_check=True,
                )
            ysb = ysbp.tile([128, 512], bf16, name="ysb", tag="ysb")
            nc.vector.tensor_copy(ysb, yps)
            nc.gpsimd.dma_start(
                y[qb * QB + qt * 128: qb * QB + (qt + 1) * 128,
                  nh * 512:(nh + 1) * 512],
                ysb,
            )


def _build(tc, xT, wq, wk, wv, wo, msk, y):
    nc = tc.nc

    with ExitStack() as top:
        singles = top.enter_context(tc.tile_pool(name="singles", bufs=1))
        QT_sb = [singles.tile([128, S], bf16, name=f"qtsb{m}", tag=f"qtsb{m}") for m in range(2)]
        KT_sb = [singles.tile([128, S], bf16, name=f"ktsb{m}", tag=f"ktsb{m}") for m in range(2)]
        V4 = singles.tile([128, NKT, HPC, HD + 1], bf16, name="v4", tag="v4")
        masks = singles.tile([128, 4 * QB], bf16, name="masks", tag="masks")
        wo_sb = singles.tile([128, 2, D], bf16, name="wo_sb", tag="wo_sb")
        nc.gpsimd.dma_start(masks, msk)
        nc.gpsimd.dma_start(wo_sb, wo.rearrange("g p c -> p g c"))
        # ones columns of the augmented V (denominator accumulators)
        nc.vector.memset(V4[:, :, :, HD:HD + 1], 1.0)

        # ---------------- projections ----------------
        with ExitStack() as proj:
            pw = proj.enter_context(tc.tile_pool(name="projw", bufs=1))
            pp = proj.enter_context(tc.tile_pool(name="projpsum", bufs=1, space="PSUM"))
            wsb = {}
            for (w, nm) in ((wq, "wq"), (wk, "wk"), (wv, "wv")):
                t = pw.tile([128, NDT, CD], bf16, name=f"{nm}sb", tag=f"{nm}sb")
                nc.sync.dma_start(t, w.rearrange("(kt p) c -> p kt c", p=128))
                wsb[nm] = t
            xsb = pw.tile([128, NDT, S], bf16, name="xsb", tag="xsb")
            xr = xT.rearrange("(kt p) s -> p kt s", p=128)
            for kt in range(NDT):
                nc.sync.dma_start(xsb[:, kt, :], xr[:, kt, :])

            # QT / KT: out[hd_block, s_chunk] = W.T @ x.T, kt-outer with 8
            # live accumulators so compute overlaps the x DMA stream
            if os.environ.get("KERNEL_ABL") == "noproj":
                return
            for (w_sb, T_sb) in ((wsb["wq"], QT_sb), (wsb["wk"], KT_sb)):
                accs = [pp.tile([128, QB], f32, name=f"ps{m}_{sc}", tag=f"ps{m}_{sc}")
                        for m in range(2) for sc in range(S // QB)]
                for kt in range(NDT):
                    for m in range(2):
                        for sc in range(S // QB):
                            nc.tensor.matmul(
                                accs[m * 4 + sc],
                                w_sb[:, kt, m * 128:(m + 1) * 128],
                                xsb[:, kt, sc * QB:(sc + 1) * QB],
                                start=(kt == 0), stop=(kt == NDT - 1),
                            )
                for m in range(2):
                    for sc in range(S // QB):
                        nc.vector.tensor_copy(T_sb[m][:, sc * QB:(sc + 1) * QB],
                                              accs[m * 4 + sc])

            # V: out[s_tile, 4*64] = x @ Wv
            for st in range(NKT):
                psv = pp.tile([128, CD], f32, name="psv", tag=f"ps0_{st % 4}")
                for kt in range(NDT):
                    nc.tensor.matmul(
                        psv,
                        xsb[:, kt, st * 128:(st + 1) * 128],
                        wsb["wv"][:, kt, :],
                        start=(kt == 0), stop=(kt == NDT - 1),
                    )
                nc.vector.tensor_copy(
                    V4[:, st, :, 0:HD],
                    psv.rearrange("p (h d) -> p h d", h=HPC),
                )

        if os.environ.get("KERNEL_ABL") == "projonly":
            return
        # ---------------- attention + output projection ----------------
        with ExitStack() as att:
            stp = att.enter_context(tc.tile_pool(name="stp", bufs=2, space="PSUM"))
            accp = att.enter_context(tc.tile_pool(name="accp", bufs=2, space="PSUM"))
            ypsp = att.enter_context(tc.tile_pool(name="ypsp", bufs=2, space="PSUM"))
            ep = att.enter_context(tc.tile_pool(name="ep", bufs=8))
            rcp = att.enter_context(tc.tile_pool(name="rcp", bufs=4))
            bcp = att.enter_context(tc.tile_pool(name="bcp", bufs=4))
            ctxnp = att.enter_context(tc.tile_pool(name="ctxnp", bufs=2))
            ysbp = att.enter_context(tc.tile_pool(name="ysbp", bufs=6))
            pools = (stp, accp, ep, rcp, bcp, ctxnp)

            noout = os.environ.get("KERNEL_ABL") == "noout"
            prev = None
            for qb in range(S // QB):
                cn2s = _attention_qb(nc, qb, QT_sb, KT_sb, V4, masks, pools)
                if prev is not None and not noout:
                    _outproj_qb(nc, qb - 1, prev, wo_sb, y, ypsp, ysbp)
                prev = cn2s
            if not noout:
                _outproj_qb(nc, S // QB - 1, prev, wo_sb, y, ypsp, ysbp)


def build_bass(reps=1):
    nc = bacc.Bacc("TRN2", target_bir_lowering=False, debug=False,
                   num_devices=NCORES)
    xT = nc.dram_tensor("xt", [D, S], bf16, kind="ExternalInput").ap()
    wq = nc.dram_tensor("wq", [D, CD], bf16, kind="ExternalInput").ap()
    wk = nc.dram_tensor("wk", [D, CD], bf16, kind="ExternalInput").ap()
    wv = nc.dram_tensor("wv", [D, CD], bf16, kind="ExternalInput").ap()
    wo = nc.dram_tensor("wo", [2, 128, D], bf16, kind="ExternalInput").ap()
    msk = nc.dram_tensor("msk", [128, 4 * QB], bf16, kind="ExternalInput").ap()
    y = nc.dram_tensor("y", [S, D], bf16, kind="ExternalOutput").ap()
    with tile.TileContext(nc) as tc:
        for _ in range(reps):
            _build(tc, xT, wq, wk, wv, wo, msk, y)
    nc.compile()
    return nc


import ml_dtypes

BF = ml_dtypes.bfloat16


def _causal_masks():
    # masks[k, rel*QB + q] = 1.0 iff rel*128 + k <= q   (rel = k-tile index
    # inside the q block)
    k = np.arange(128)[:, None]
    q = np.arange(QB)[None, :]
    cols = [(rel * 128 + k <= q).astype(BF) for rel in range(4)]
    return np.concatenate(cols, axis=1)


def make_in_maps(x, Wq, Wk, Wv, Wo):
    msk = _causal_masks()
    in_maps = []
    for c in range(NCORES):
        b, g = divmod(c, 4)
        cs = slice(g * CD, (g + 1) * CD)
        in_maps.append({
            "xt": np.ascontiguousarray(x[b].T).astype(BF),
            "wq": np.ascontiguousarray(Wq[:, cs]).astype(BF),
            "wk": np.ascontiguousarray(Wk[:, cs]).astype(BF),
            "wv": np.ascontiguousarray(Wv[:, cs]).astype(BF),
            "wo": np.ascontiguousarray(Wo[cs, :]).reshape(2, 128, D).astype(BF),
            "msk": msk,
        })
    return in_maps


_NC_CACHE = None


def get_nc():
    global _NC_CACHE
    if _NC_CACHE is None:
        _NC_CACHE = build_bass()
    return _NC_CACHE


def kernel(x, Wq, Wk, Wv, Wo, trace=False, **trace_kwargs):
    x = np.asarray(x, dtype=np.float32)
    in_maps = make_in_maps(x, np.asarray(Wq, np.float32), np.asarray(Wk, np.float32),
                           np.asarray(Wv, np.float32), np.asarray(Wo, np.float32))
    res = run_bass_kernel_spmd(get_nc(), in_maps, core_ids=list(range(NCORES)),
                               trace=trace, **trace_kwargs)
    parts = [np.asarray(r["y"], dtype=np.float32) for r in res.results]
    out = np.empty((B, S, D), dtype=np.float32)
    for b in range(B):
        out[b] = parts[4 * b] + parts[4 * b + 1] + parts[4 * b + 2] + parts[4 * b + 3]
    kernel.last_results = res
    return out
